# revision 32
# baseline (speedup 1.0000x reference)
"""GQA attention kernel for Trainium2, 8-core tensor-parallel (by heads).

Shapes (hardcoded from the problem spec):
  x:(4,128,4096) fp32, wq:(4096,4096), wk/wv:(4096,1024), wo:(4096,4096),
  32 q heads / 8 kv heads, head_dim 128, start_pos=0 (cache is overwritten).

Sharding: core c owns q heads [4c,4c+4) and kv head c; wq/wk/wv column-
sharded, wo row-sharded; each core computes a full (512,4096) partial of
the output projection; host sums the 8 partials and adds bo.

Device-side design (bf16 matmul path, fp32 accumulation):
  - Q/K projections computed feature-major (weights stationary, tokens
    moving) so attention needs no transposes.  Q/K features are permuted
    on the host to [evens, odds] within each head so RoPE is 6 DVE ops
    on contiguous partition halves.
  - Biases folded in as a 33rd contraction chunk (x row of ones, bias
    row in the weight pack).
  - All inputs repacked on the host into [128, k*cols] slabs so each
    array loads with O(1) large DMAs.
  - Softmax denominator via ones-matmul; 1/denom folded into the AV
    epilogue multiply.
"""
import sys
sys.path.insert(0, "/opt/trn_rl_repo")

import numpy as np
from ml_dtypes import bfloat16

B, S, D = 4, 128, 4096
H, KV, HD = 32, 8, 128
NCORES = 8
HQ = H // NCORES          # 4 q heads per core
T = B * S                 # 512 tokens
NK = D // 128 + 1         # 32 k-chunks + 1 bias chunk
QF = HQ * HD              # 512 q features per core
WQK = QF + HD             # 640 = q features + k features
SCALE = 1.0 / float(np.sqrt(HD))
PIECES = (4, 8, 7, 7, 7)  # k-chunks per phase-A DMA piece (small first piece
                          # so the PE starts as early as possible)

_CACHE = {}


def _build():
    import concourse.tile as tile
    from concourse import bacc, mybir

    F32, BF16 = mybir.dt.float32, mybir.dt.bfloat16
    AF = mybir.ActivationFunctionType

    nc = bacc.Bacc("TRN2", target_bir_lowering=False, debug=False,
                   enable_asserts=False, num_devices=NCORES)

    xt_d = nc.dram_tensor("xt", [128, NK * T], BF16, kind="ExternalInput").ap()
    wqk_d = nc.dram_tensor("wqk", [128, NK * WQK], BF16, kind="ExternalInput").ap()
    wv_d = nc.dram_tensor("wv", [128, NK * HD], BF16, kind="ExternalInput").ap()
    wo_d = nc.dram_tensor("wo", [128, HQ * D], BF16, kind="ExternalInput").ap()
    cosT_d = nc.dram_tensor("cosT", [128, T], BF16, kind="ExternalInput").ap()
    sinT_d = nc.dram_tensor("sinT", [128, T], BF16, kind="ExternalInput").ap()
    mkT_d = nc.dram_tensor("mkT", [128, T], BF16, kind="ExternalInput").ap()
    on_d = nc.dram_tensor("on", [128, S], BF16, kind="ExternalInput").ap()
    id_d = nc.dram_tensor("idm", [128, S], BF16, kind="ExternalInput").ap()
    out_d = nc.dram_tensor("out", [T, D], BF16, kind="ExternalOutput").ap()

    P0 = [0]
    for p in PIECES:
        P0.append(P0[-1] + p)

    with tile.TileContext(nc) as tc:
        with tc.tile_pool(name="w", bufs=1) as wp, \
             tc.tile_pool(name="consts", bufs=1) as cp, \
             tc.tile_pool(name="qk", bufs=1) as qkp, \
             tc.tile_pool(name="rt", bufs=4) as rtp, \
             tc.tile_pool(name="attn", bufs=2) as ap_, \
             tc.tile_pool(name="aop", bufs=1) as aop, \
             tc.tile_pool(name="outp", bufs=6) as op:

            # ---- input DMAs: phase-A slabs first, split across both rings.
            # wv rides right behind piece 0 (piece-0 V matmuls need it).
            xt_p, wqk_p = [], []
            wv_t = wp.tile([128, NK * HD], BF16, name="wv")
            for i, npc in enumerate(PIECES):
                xt_t = wp.tile([128, npc * T], BF16, name=f"xtp{i}")
                wq_t = wp.tile([128, npc * WQK], BF16, name=f"wqkp{i}")
                e1, e2 = (nc.sync, nc.scalar) if i % 2 == 0 else (nc.scalar, nc.sync)
                e1.dma_start(xt_t, xt_d[:, P0[i] * T:P0[i + 1] * T])
                e2.dma_start(wq_t, wqk_d[:, P0[i] * WQK:P0[i + 1] * WQK])
                if i == 0:
                    nc.sync.dma_start(wv_t, wv_d)
                xt_p.append(xt_t)
                wqk_p.append(wq_t)
            # cos/sin replicated into both partition halves so every
            # two-SBUF-operand DVE op has base-partition-aligned inputs.
            cosT = cp.tile([128, T], BF16)
            nc.scalar.dma_start(cosT, cosT_d)
            sinT = cp.tile([128, T], BF16)
            nc.scalar.dma_start(sinT, sinT_d)
            mkT = cp.tile([128, T], BF16)
            nc.sync.dma_start(mkT, mkT_d)
            on128 = cp.tile([128, S], BF16)
            nc.scalar.dma_start(on128, on_d)
            ident = cp.tile([128, S], BF16)
            nc.sync.dma_start(ident, id_d)
            wo_p = []
            for h in range(HQ):
                wo_t = wp.tile([128, D], BF16, name=f"wop{h}")
                (nc.sync if h % 2 == 0 else nc.scalar).dma_start(
                    wo_t, wo_d[:, h * D:(h + 1) * D])
                wo_p.append(wo_t)

            qb = [qkp.tile([128, T], BF16, name=f"qb{h}") for h in range(HQ)]
            kb = qkp.tile([128, T], BF16, name="kb")
            vb = qkp.tile([128, T], BF16, name="vb")
            ao = [aop.tile([128, T], BF16, name=f"ao{h}") for h in range(HQ)]

            # ---------------- Phase A: QKV projections (all feature-major)
            with tc.tile_pool(name="psA", bufs=1, space="PSUM") as psA:
                psq = [psA.tile([128, T], F32, name=f"psq{h}") for h in range(HQ)]
                psk = psA.tile([128, T], F32, name="psk")
                psv = psA.tile([128, T], F32, name="psv")
                def xk_of(k):
                    pi = 0
                    while k >= P0[pi + 1]:
                        pi += 1
                    lk = k - P0[pi]
                    return (xt_p[pi][:, lk * T:(lk + 1) * T],
                            wqk_p[pi][:, lk * WQK:(lk + 1) * WQK])

                # K/Q0/V accumulate piece-by-piece and close as soon as the
                # last DMA piece lands (~1/3 into the PE work); Q1-Q3 follow.
                # RoPE and the head-0 attention chain then overlap the
                # remaining Q-head matmuls instead of waiting for all of
                # phase A.
                def lhs_of(t, k):
                    _, wk_ = xk_of(k)
                    if t == 0:
                        return wk_[:, QF:QF + HD]
                    if t <= HQ:
                        return wk_[:, (t - 1) * HD:t * HD]
                    return wv_t[:, k * HD:(k + 1) * HD]

                def emit(t, tgt, k):
                    xk, _ = xk_of(k)
                    nc.tensor.matmul(tgt, lhs_of(t, k), xk,
                                     start=(k == 0), stop=(k == NK - 1))

                for pi in range(len(PIECES)):
                    for t, tgt in ((0, psk), (1, psq[0]), (HQ + 1, psv)):
                        for k in range(P0[pi], P0[pi + 1]):
                            emit(t, tgt, k)
                for h in range(1, HQ):
                    for k in range(NK):
                        emit(h + 1, psq[h], k)

                # ---- RoPE (feature-permuted: evens rows 0-63, odds 64-127)
                # Stage PSUM->SBUF bf16 on the ACT engine first; the six
                # rotate ops then run all-bf16-SBUF, hitting the DVE 4x mode.
                for src, dst in [(psk, kb)] + [(psq[h], qb[h]) for h in range(HQ)]:
                    sf = rtp.tile([128, T], BF16, tag="sf", bufs=3)
                    nc.scalar.copy(sf, src)
                    e, o = sf[0:64, :], sf[64:128, :]
                    t1 = rtp.tile([64, T], BF16, tag="t1")
                    t2 = rtp.tile([64, T], BF16, tag="t2")
                    nc.vector.tensor_mul(t1, o, sinT[64:128, :])
                    nc.vector.tensor_mul(t2, e, cosT[0:64, :])
                    nc.vector.tensor_sub(dst[0:64, :], t2, t1)
                    t3 = rtp.tile([64, T], BF16, tag="t1")
                    t4 = rtp.tile([64, T], BF16, tag="t2")
                    nc.vector.tensor_mul(t3, o, cosT[64:128, :])
                    nc.vector.tensor_mul(t4, e, sinT[0:64, :])
                    nc.vector.tensor_add(dst[64:128, :], t4, t3)
                # V was computed feature-major ([vf, tok]); transpose each
                # batch block through the PE to get token-major vb for AV.
                vfm = qkp.tile([128, T], BF16, name="vfm")
                nc.vector.tensor_copy(vfm, psv)
                for m in range(B):
                    pvT = psA.tile([128, S], BF16, tag="pvT", bufs=2,
                                   name=f"pvT{m}")
                    nc.tensor.transpose(pvT, vfm[:, m * S:(m + 1) * S], ident)
                    nc.vector.tensor_copy(vb[:, m * S:(m + 1) * S], pvT)

            # ---------------- Attention (per q head; layouts [j, i])
            with tc.tile_pool(name="psB", bufs=2, space="PSUM") as psB:
                for h in range(HQ):
                    psS = psB.tile([128, T], F32, tag="psS", name=f"psS{h}", bufs=2)
                    for m in range(B):
                        sl = slice(m * S, (m + 1) * S)
                        nc.tensor.matmul(psS[:, sl], kb[:, sl], qb[h][:, sl],
                                         start=True, stop=True)
                    au = ap_.tile([128, T], BF16, tag="au", name=f"au{h}")
                    nc.scalar.activation(au, psS, AF.Exp, scale=SCALE)
                    au2 = ap_.tile([128, T], BF16, tag="au2", name=f"au2{h}")
                    nc.vector.tensor_mul(au2, au, mkT)
                    pden = psB.tile([128, T], F32, tag="pden", name=f"pden{h}", bufs=1)
                    nc.tensor.matmul(pden, on128, au2, start=True, stop=True)
                    rec = ap_.tile([128, T], F32, tag="rec", name=f"rec{h}")
                    nc.vector.reciprocal(rec, pden)
                    psO = psB.tile([128, T], F32, tag="psO", name=f"psO{h}", bufs=1)
                    for m in range(B):
                        sl = slice(m * S, (m + 1) * S)
                        nc.tensor.matmul(psO[:, sl], vb[:, sl], au2[:, sl],
                                         start=True, stop=True)
                    nc.vector.tensor_mul(ao[h], psO, rec)

                # ---------------- Output projection
                # Out DMAs are issued per (m, n) tile on alternating rings so
                # the writeback fully overlaps the projection matmuls instead
                # of serializing into a tail on one ring.
                NT = D // 512
                for m in range(B):
                    for n in range(NT):
                        pso = psB.tile([128, 512], F32, tag="pso", bufs=4,
                                       name=f"pso{m}_{n}")
                        for h in range(HQ):
                            nc.tensor.matmul(pso,
                                             ao[h][:, m * S:(m + 1) * S],
                                             wo_p[h][:, n * 512:(n + 1) * 512],
                                             start=(h == 0), stop=(h == HQ - 1))
                        osb = op.tile([128, 512], BF16, tag="osb",
                                      name=f"osb{m}_{n}")
                        if (m * NT + n) % 2 == 0:
                            nc.vector.tensor_copy(osb, pso)
                        else:
                            nc.scalar.copy(osb, pso)
                        eng = nc.sync if (m * NT + n) % 2 == 0 else nc.scalar
                        eng.dma_start(
                            out_d[m * S:(m + 1) * S, n * 512:(n + 1) * 512], osb)

    nc.compile()
    return nc


_PERM = np.concatenate([np.arange(0, HD, 2), np.arange(1, HD, 2)])


def _prep_inputs(x, freqs_cos, freqs_sin, wq, bq, wk, bk, wv, bv, wo):
    bf = bfloat16
    xT = np.asarray(x, np.float32).reshape(T, D).T          # [D, T]
    xt_all = np.zeros((NK, 128, T), np.float32)
    xt_all[:NK - 1] = xT.reshape(NK - 1, 128, T)
    xt_all[NK - 1, 0, :] = 1.0
    xt_packed = np.ascontiguousarray(
        xt_all.transpose(1, 0, 2).reshape(128, NK * T)).astype(bf)
    cosT = np.ascontiguousarray(
        np.tile(np.asarray(freqs_cos, np.float32).T, (2, B))).astype(bf)
    sinT = np.ascontiguousarray(
        np.tile(np.asarray(freqs_sin, np.float32).T, (2, B))).astype(bf)
    mkT = np.ascontiguousarray(
        np.tile(np.triu(np.ones((S, S), np.float32)), (1, B))).astype(bf)
    on = np.ones((128, S), np.float32).astype(bf)
    idm = np.eye(S, dtype=np.float32).astype(bf)
    wqf = np.asarray(wq, np.float32)
    bqf = np.asarray(bq, np.float32)
    wkf = np.asarray(wk, np.float32)
    bkf = np.asarray(bk, np.float32)
    wvf = np.asarray(wv, np.float32)
    bvf = np.asarray(bv, np.float32)
    wof = np.asarray(wo, np.float32)
    maps = []
    for c in range(NCORES):
        qs = slice(c * QF, (c + 1) * QF)
        ks = slice(c * HD, (c + 1) * HD)
        wq_c = wqf[:, qs].reshape(D, HQ, HD)[:, :, _PERM].reshape(D, QF)
        bq_c = bqf[qs].reshape(HQ, HD)[:, _PERM].reshape(QF)
        wk_c = wkf[:, ks][:, _PERM]
        bk_c = bkf[ks][_PERM]
        wqk = np.concatenate([wq_c, wk_c], axis=1)          # [D, 640]
        bqk = np.concatenate([bq_c, bk_c])
        wqk_all = np.zeros((NK, 128, WQK), np.float32)
        wqk_all[:NK - 1] = wqk.reshape(NK - 1, 128, WQK)
        wqk_all[NK - 1, 0, :] = bqk
        wqk_packed = np.ascontiguousarray(
            wqk_all.transpose(1, 0, 2).reshape(128, NK * WQK)).astype(bf)
        wv_all = np.zeros((NK, 128, HD), np.float32)
        wv_all[:NK - 1] = wvf[:, ks].reshape(NK - 1, 128, HD)
        wv_all[NK - 1, 0, :] = bvf[ks]
        wv_packed = np.ascontiguousarray(
            wv_all.transpose(1, 0, 2).reshape(128, NK * HD)).astype(bf)
        wo_packed = np.ascontiguousarray(
            wof[qs, :].reshape(HQ, 128, D).transpose(1, 0, 2)
            .reshape(128, HQ * D)).astype(bf)
        maps.append({
            "xt": xt_packed, "wqk": wqk_packed, "wv": wv_packed,
            "wo": wo_packed, "cosT": cosT, "sinT": sinT, "mkT": mkT, "on": on,
            "idm": idm,
        })
    return maps


def kernel(x, start_pos, freqs_cos, freqs_sin, mask, cache_k, cache_v,
           wq, bq, wk, bk, wv, bv, wo, bo):
    from concourse.bass_utils import run_bass_kernel_spmd

    assert int(start_pos) == 0
    if "nc" not in _CACHE:
        _CACHE["nc"] = _build()
    nc = _CACHE["nc"]
    in_maps = _prep_inputs(np.asarray(x), np.asarray(freqs_cos),
                           np.asarray(freqs_sin), np.asarray(wq),
                           np.asarray(bq), np.asarray(wk), np.asarray(bk),
                           np.asarray(wv), np.asarray(bv), np.asarray(wo))
    res = run_bass_kernel_spmd(nc, in_maps, core_ids=list(range(NCORES)))
    acc = np.zeros((T, D), np.float64)
    for r in res.results:
        acc += r["out"].astype(np.float64)
    out = (acc + np.asarray(bo).astype(np.float64)).astype(np.float32)
    return out.reshape(B, S, D)
